# revision 1
# baseline (speedup 1.0000x reference)
"""GraphSAGE sim/cor dual-branch GNN on 8 Trainium2 NeuronCores.

Sharding: dst-node partition across 8 cores (per sharding hint). Host does
index preprocessing only (edge bucketing by dst shard, sort-by-dst, padding,
count/reciprocal tables, dtype packing); all FP tensor compute (embedding
sums aggregation, segment mean via one-hot matmul on PE, FC layers, final
mixing matmuls) runs on device via Bass/Tile kernels.

Math reformulation (linearity of mean-aggregation):
  layer0: u[d] = g[d] + mean_{e->d} g[src_e]; h0 = relu(u @ W_in + b_in*(1+[cnt>0]))
  layer1: out[d] = p[d] + mean_{e->d} p[src_e] + b_out, with p = h0 @ W_out
  (bias handled exactly via augmented feature column; W_in applied after
   aggregation - 144-dim rows gathered instead of 256-dim)
"""
import os
import numpy as np
import ml_dtypes

N0, N1, N2 = 200000, 50000, 10000
HID, OUT = 256, 128
DG = 144          # concat embedding dim
DGA = DG + 1      # augmented with bias/mask column
NC = 8
S1 = N1 // NC     # 6250 dst per core, layer0
S2 = N2 // NC     # 1250 dst per core, layer1
T1 = (S1 + 127) // 128   # 49 tiles
T2 = (S2 + 127) // 128   # 10 tiles

_exec_times = []


def _pack_edges(src, dst, n_dst_shard, n_tiles, ect):
    """Sort edges by dst, bucket into 128-dst tiles, pad tile t to ect[t]
    chunks of 128 edge slots. Returns eidx [128, sum(ect)] int32 (src ids),
    dstl [128, sum(ect)] f32 (dst-local-in-tile, -1 for pad)."""
    order = np.argsort(dst, kind="stable")
    src = src[order]
    dst = dst[order]
    tid = dst // 128
    starts = np.concatenate([[0], np.cumsum(ect)])
    ctot = int(starts[-1])
    eidx = np.zeros((ctot * 128,), np.int32)
    dstl = np.full((ctot * 128,), -1.0, np.float32)
    bounds = np.searchsorted(tid, np.arange(n_tiles + 1))
    for t in range(n_tiles):
        a, b = bounds[t], bounds[t + 1]
        base = int(starts[t]) * 128
        n = b - a
        eidx[base:base + n] = src[a:b]
        dstl[base:base + n] = (dst[a:b] - t * 128).astype(np.float32)
    eidx = eidx.reshape(ctot, 128).T.copy()
    dstl = dstl.reshape(ctot, 128).T.copy()
    return eidx, dstl


def _shard_prep(e_src, e_dst, shard, n_tiles):
    """Per-core edge lists (dst in shard) -> locals + counts."""
    lists = []
    for c in range(NC):
        m = (e_dst >= c * shard) & (e_dst < (c + 1) * shard)
        lists.append((e_src[m], e_dst[m] - c * shard))
    ect = np.ones(n_tiles, np.int64)
    for s, d in lists:
        cnt = np.bincount(d // 128, minlength=n_tiles)
        ect = np.maximum(ect, (cnt + 127) // 128)
    return lists, ect


def _recips(dst_local, shard, n_tiles):
    cnt = np.bincount(dst_local, minlength=n_tiles * 128).astype(np.float32)
    recip = 1.0 / np.maximum(cnt, 1.0)
    mask1p = 1.0 + (cnt > 0)
    return (recip.reshape(n_tiles, 128).T.copy(),
            mask1p.astype(np.float32).reshape(n_tiles, 128).T.copy())


def _build_launch_a(ect0):
    import concourse.bass as bass
    import concourse.bacc as bacc
    import concourse.mybir as mybir
    import concourse.tile as tile

    bf16 = mybir.dt.bfloat16
    f32 = mybir.dt.float32
    nc = bacc.Bacc(enable_partition_id=False)
    C0 = int(ect0.sum())
    starts0 = np.concatenate([[0], np.cumsum(ect0)]).astype(int)
    g = {}
    for br in ("sim", "cor"):
        g[br] = dict(
            gtab=nc.dram_tensor(f"gtab_{br}", [N0, DG], bf16, kind="ExternalInput"),
            eidx=nc.dram_tensor(f"eidx_{br}", [128, C0], mybir.dt.int32, kind="ExternalInput"),
            dstl=nc.dram_tensor(f"dstl_{br}", [128, C0], f32, kind="ExternalInput"),
            recip=nc.dram_tensor(f"recip_{br}", [128, T1], f32, kind="ExternalInput"),
            mask1p=nc.dram_tensor(f"mask1p_{br}", [128, T1], f32, kind="ExternalInput"),
            win=nc.dram_tensor(f"win_{br}", [DGA, HID], bf16, kind="ExternalInput"),
            wout=nc.dram_tensor(f"wout_{br}", [HID, OUT], bf16, kind="ExternalInput"),
            gself=nc.dram_tensor(f"gself_{br}", [128, T1 * DG], bf16, kind="ExternalInput"),
            pt=nc.dram_tensor(f"pt_{br}", [128, T1 * 128], f32, kind="ExternalOutput"),
        )
    iota_in = nc.dram_tensor("iota", [128, 128], f32, kind="ExternalInput")
    ident_in = nc.dram_tensor("ident", [128, 128], f32, kind="ExternalInput")

    with tile.TileContext(nc) as tc:
        with tc.tile_pool(name="const", bufs=1) as cp, \
             tc.tile_pool(name="gath", bufs=8) as gp, \
             tc.tile_pool(name="work", bufs=3) as wp, \
             tc.tile_pool(name="pagg", bufs=2, space="PSUM") as pagg, \
             tc.tile_pool(name="pfc", bufs=1, space="PSUM") as pfc:
            iota = cp.tile([128, 128], f32)
            nc.sync.dma_start(out=iota[:], in_=iota_in[:])
            ident = cp.tile([128, 128], f32)
            nc.sync.dma_start(out=ident[:], in_=ident_in[:])
            for br in ("sim", "cor"):
                tt = g[br]
                eidx_t = cp.tile([128, C0], mybir.dt.int32, tag=f"ei{br}")
                nc.sync.dma_start(out=eidx_t[:], in_=tt["eidx"][:])
                dstl_t = cp.tile([128, C0], f32, tag=f"dl{br}")
                nc.sync.dma_start(out=dstl_t[:], in_=tt["dstl"][:])
                recip_t = cp.tile([128, T1], f32, tag=f"rc{br}")
                nc.sync.dma_start(out=recip_t[:], in_=tt["recip"][:])
                mask_t = cp.tile([128, T1], f32, tag=f"mk{br}")
                nc.sync.dma_start(out=mask_t[:], in_=tt["mask1p"][:])
                win_t = cp.tile([128, 2 * HID], bf16, tag=f"wi{br}")  # rows 0:128 | 128:145 padded
                nc.sync.dma_start(out=win_t[:, :HID], in_=tt["win"][0:128, :])
                nc.sync.dma_start(out=win_t[:DGA - 128, HID:], in_=tt["win"][128:DGA, :])
                wout_t = cp.tile([128, 2 * OUT], bf16, tag=f"wo{br}")
                nc.sync.dma_start(out=wout_t[:, :OUT], in_=tt["wout"][0:128, :])
                nc.sync.dma_start(out=wout_t[:, OUT:], in_=tt["wout"][128:HID, :])

                for t in range(T1):
                    agg = pagg.tile([128, DG], f32, tag="agg")
                    nch = int(ect0[t])
                    for j in range(nch):
                        col = int(starts0[t]) + j
                        m = gp.tile([128, DG], bf16, tag="m")
                        nc.gpsimd.indirect_dma_start(
                            out=m[:], out_offset=None, in_=tt["gtab"][:],
                            in_offset=bass.IndirectOffsetOnAxis(
                                ap=eidx_t[:, col:col + 1], axis=0))
                        oh = wp.tile([128, 128], bf16, tag="oh")
                        nc.vector.tensor_tensor(
                            out=oh[:], in0=dstl_t[:, col:col + 1].to_broadcast([128, 128]),
                            in1=iota[:], op=mybir.AluOpType.is_equal)
                        nc.tensor.matmul(agg[:], lhsT=oh[:], rhs=m[:],
                                         start=(j == 0), stop=(j == nch - 1))
                    gs = wp.tile([128, DG], bf16, tag="gs")
                    nc.sync.dma_start(out=gs[:], in_=tt["gself"][:, t * DG:(t + 1) * DG])
                    u = wp.tile([128, DGA], f32, tag="u")
                    nc.vector.tensor_scalar_mul(u[:, :DG], agg[:], recip_t[:, t:t + 1])
                    nc.vector.tensor_add(u[:, :DG], u[:, :DG], gs[:])
                    nc.vector.tensor_copy(out=u[:, DG:DGA], in_=mask_t[:, t:t + 1])
                    uta_p = pfc.tile([128, 128], f32, tag="uta")
                    nc.tensor.transpose(out=uta_p[:], in_=u[:, :128], identity=ident[:])
                    utb_p = pfc.tile([32, 128], f32, tag="utb")
                    nc.tensor.transpose(out=utb_p[:DGA - 128, :], in_=u[:, 128:DGA],
                                        identity=ident[:])
                    uta = wp.tile([128, 128], bf16, tag="uta_s")
                    nc.vector.tensor_copy(out=uta[:], in_=uta_p[:])
                    utb = wp.tile([32, 128], bf16, tag="utb_s")
                    nc.vector.tensor_copy(out=utb[:DGA - 128, :], in_=utb_p[:DGA - 128, :])
                    h0 = wp.tile([128, 2 * 128], bf16, tag="h0")
                    for half in range(2):
                        fc = pfc.tile([128, 128], f32, tag=f"fc{half}")
                        nc.tensor.matmul(fc[:], lhsT=win_t[:, half * 128:half * 128 + 128],
                                         rhs=uta[:], start=True, stop=False)
                        nc.tensor.matmul(fc[:], lhsT=win_t[:DGA - 128,
                                                          HID + half * 128:HID + half * 128 + 128],
                                         rhs=utb[:DGA - 128, :], start=False, stop=True)
                        nc.scalar.activation(out=h0[:, half * 128:(half + 1) * 128], in_=fc[:],
                                             func=mybir.ActivationFunctionType.Relu)
                    pt_p = pfc.tile([128, 128], f32, tag="pt")
                    nc.tensor.matmul(pt_p[:], lhsT=wout_t[:, :OUT], rhs=h0[:, :128],
                                     start=True, stop=False)
                    nc.tensor.matmul(pt_p[:], lhsT=wout_t[:, OUT:], rhs=h0[:, 128:],
                                     start=False, stop=True)
                    pt_s = wp.tile([128, 128], f32, tag="pt_s")
                    nc.vector.tensor_copy(out=pt_s[:], in_=pt_p[:])
                    nc.sync.dma_start(out=tt["pt"][:, t * 128:(t + 1) * 128], in_=pt_s[:])
    nc.compile()
    return nc


def _build_launch_b(ect1, coef):
    import concourse.bass as bass
    import concourse.bacc as bacc
    import concourse.mybir as mybir
    import concourse.tile as tile

    f32 = mybir.dt.float32
    nc = bacc.Bacc(enable_partition_id=False)
    C1 = int(ect1.sum())
    starts1 = np.concatenate([[0], np.cumsum(ect1)]).astype(int)
    a1, a2, b2 = coef
    g = {}
    for br in ("sim", "cor"):
        g[br] = dict(
            ptab=nc.dram_tensor(f"ptab_{br}", [N1, OUT], f32, kind="ExternalInput"),
            eidx=nc.dram_tensor(f"eidx_{br}", [128, C1], mybir.dt.int32, kind="ExternalInput"),
            dstl=nc.dram_tensor(f"dstl_{br}", [128, C1], f32, kind="ExternalInput"),
            rpe=nc.dram_tensor(f"rpe_{br}", [128, C1], f32, kind="ExternalInput"),
            ptself=nc.dram_tensor(f"ptself_{br}", [128, T2 * 128], f32, kind="ExternalInput"),
            zt=nc.dram_tensor(f"zt_{br}", [128, T2 * 128], f32, kind="ExternalOutput"),
        )
    wcs_in = nc.dram_tensor("wcs", [OUT, OUT], f32, kind="ExternalInput")
    wsc_in = nc.dram_tensor("wsc", [OUT, OUT], f32, kind="ExternalInput")
    bo_in = nc.dram_tensor("bo", [128, 2], f32, kind="ExternalInput")
    iota_in = nc.dram_tensor("iota", [128, 128], f32, kind="ExternalInput")
    ones_in = nc.dram_tensor("ones", [1, 128], f32, kind="ExternalInput")

    with tile.TileContext(nc) as tc:
        with tc.tile_pool(name="const", bufs=1) as cp, \
             tc.tile_pool(name="gath", bufs=8) as gp, \
             tc.tile_pool(name="work", bufs=3) as wp, \
             tc.tile_pool(name="pagg", bufs=2, space="PSUM") as pagg, \
             tc.tile_pool(name="pmix", bufs=1, space="PSUM") as pmix:
            iota = cp.tile([128, 128], f32)
            nc.sync.dma_start(out=iota[:], in_=iota_in[:])
            ones = cp.tile([1, 128], f32)
            nc.sync.dma_start(out=ones[:], in_=ones_in[:])
            wcs = cp.tile([128, OUT], f32)
            nc.sync.dma_start(out=wcs[:], in_=wcs_in[:])
            wsc = cp.tile([128, OUT], f32)
            nc.sync.dma_start(out=wsc[:], in_=wsc_in[:])
            bo = cp.tile([128, 2], f32)
            nc.sync.dma_start(out=bo[:], in_=bo_in[:])
            tiles = {}
            for br in ("sim", "cor"):
                tt = g[br]
                eidx_t = cp.tile([128, C1], mybir.dt.int32, tag=f"ei{br}")
                nc.sync.dma_start(out=eidx_t[:], in_=tt["eidx"][:])
                dstl_t = cp.tile([128, C1], f32, tag=f"dl{br}")
                nc.sync.dma_start(out=dstl_t[:], in_=tt["dstl"][:])
                ptself_t = cp.tile([128, T2 * 128], f32, tag=f"ps{br}")
                nc.sync.dma_start(out=ptself_t[:], in_=tt["ptself"][:])
                rpe_t = cp.tile([128, C1], f32, tag=f"rp{br}")
                nc.sync.dma_start(out=rpe_t[:], in_=tt["rpe"][:])
                tiles[br] = (eidx_t, dstl_t, ptself_t, rpe_t)
            for t in range(T2):
                br_out = {}
                for bi, br in enumerate(("sim", "cor")):
                    tt = g[br]
                    eidx_t, dstl_t, ptself_t, rpe_t = tiles[br]
                    aggp = pagg.tile([128, 128], f32, tag="aggp")
                    nch = int(ect1[t])
                    for j in range(nch):
                        col = int(starts1[t]) + j
                        mp = gp.tile([128, OUT], f32, tag="mp")
                        nc.gpsimd.indirect_dma_start(
                            out=mp[:], out_offset=None, in_=tt["ptab"][:],
                            in_offset=bass.IndirectOffsetOnAxis(
                                ap=eidx_t[:, col:col + 1], axis=0))
                        nc.vector.tensor_scalar_mul(mp[:], mp[:], rpe_t[:, col:col + 1])
                        oh = wp.tile([128, 128], f32, tag="oh")
                        nc.vector.tensor_tensor(
                            out=oh[:], in0=dstl_t[:, col:col + 1].to_broadcast([128, 128]),
                            in1=iota[:], op=mybir.AluOpType.is_equal)
                        nc.tensor.matmul(aggp[:], lhsT=mp[:], rhs=oh[:],
                                         start=(j == 0), stop=(j == nch - 1))
                    sT = wp.tile([128, 128], f32, tag=f"sT{br}")
                    nc.vector.tensor_add(sT[:], ptself_t[:, t * 128:(t + 1) * 128], aggp[:])
                    nc.vector.tensor_scalar_add(sT[:], sT[:], bo[:, bi:bi + 1])
                    br_out[br] = sT
                sT, cT = br_out["sim"], br_out["cor"]
                cs_p = pmix.tile([128, 128], f32, tag="cs")
                nc.tensor.matmul(cs_p[:], lhsT=wcs[:], rhs=cT[:], start=True, stop=True)
                cs = wp.tile([128, 128], f32, tag="css")
                nc.vector.tensor_copy(out=cs[:], in_=cs_p[:])
                sc_p = pmix.tile([128, 128], f32, tag="sc")
                nc.tensor.matmul(sc_p[:], lhsT=wsc[:], rhs=sT[:], start=True, stop=True)
                sc = wp.tile([128, 128], f32, tag="scs")
                nc.vector.tensor_copy(out=sc[:], in_=sc_p[:])
                z1c = wp.tile([128, 128], f32, tag="z1c")
                nc.scalar.mul(z1c[:], cT[:], float(1 - a1))
                t1 = wp.tile([128, 128], f32, tag="t1")
                nc.scalar.mul(t1[:], sc[:], float(a1))
                nc.vector.tensor_add(z1c[:], z1c[:], t1[:])
                z1s = wp.tile([128, 128], f32, tag="z1s")
                nc.scalar.mul(z1s[:], sT[:], float(1 - a1))
                t2_ = wp.tile([128, 128], f32, tag="t2")
                nc.scalar.mul(t2_[:], cs[:], float(a1))
                nc.vector.tensor_add(z1s[:], z1s[:], t2_[:])
                w1_p = pmix.tile([128, 128], f32, tag="w1")
                nc.tensor.matmul(w1_p[:], lhsT=wcs[:], rhs=z1c[:], start=True, stop=True)
                w2_p = pmix.tile([128, 128], f32, tag="w2")
                nc.tensor.matmul(w2_p[:], lhsT=wsc[:], rhs=z1s[:], start=True, stop=True)
                for br, base, mixv, wv in (("sim", sT, cs, w1_p), ("cor", cT, sc, w2_p)):
                    z = wp.tile([128, 128], f32, tag=f"z{br}")
                    nc.scalar.mul(z[:], base[:], float(1 - a2 - b2))
                    t3 = wp.tile([128, 128], f32, tag="t3")
                    nc.scalar.mul(t3[:], mixv[:], float(a2))
                    nc.vector.tensor_add(z[:], z[:], t3[:])
                    t4 = wp.tile([128, 128], f32, tag="t4")
                    nc.scalar.mul(t4[:], wv[:], float(b2))
                    nc.vector.tensor_add(z[:], z[:], t4[:])
                    nc.sync.dma_start(out=g[br]["zt"][:, t * 128:(t + 1) * 128], in_=z[:])
    nc.compile()
    return nc


def kernel(**inputs):
    from concourse.bass_utils import run_bass_kernel_spmd
    global _exec_times
    _exec_times = []
    trace = os.environ.get("BASS_KERNEL_TRACE", "0") == "1"
    tkw = {}
    if trace:
        import sys, types
        import antenv
        from trn_agent_boot.trn_boot import _ntff_profile_via_ctypes
        if "antenv.axon_hooks" not in sys.modules:
            mod = types.ModuleType("antenv.axon_hooks")
            mod.get_axon_ntff_profile_hook = (
                lambda: _ntff_profile_via_ctypes("/opt/axon/libaxon_pjrt.so"))
            mod.set_axon_ntff_profile_hook = lambda h: None
            sys.modules["antenv.axon_hooks"] = mod
            antenv.axon_hooks = mod

    bf16 = ml_dtypes.bfloat16
    x = np.asarray(inputs["x"]).astype(np.int64)
    branches = {}
    for br in ("sim", "cor"):
        tabs = [np.asarray(inputs[f"emb_{br}_{i}"], np.float32) for i in range(5)]
        gtab = np.concatenate([tabs[i][x[:, i]] for i in range(5)], axis=1)
        win = np.asarray(inputs[f"W_in_{br}"], np.float32)
        bin_ = np.asarray(inputs[f"b_in_{br}"], np.float32)
        win_aug = np.concatenate([win, bin_[None, :]], 0)
        branches[br] = dict(
            gtab=np.ascontiguousarray(gtab.astype(bf16)),
            win=win_aug.astype(bf16),
            wout=np.asarray(inputs[f"W_out_{br}"], np.float32).astype(bf16),
            bout=np.asarray(inputs[f"b_out_{br}"], np.float32),
            e0s=np.asarray(inputs[f"e0_{br}_src"]).astype(np.int64),
            e0d=np.asarray(inputs[f"e0_{br}_dst"]).astype(np.int64),
            e1s=np.asarray(inputs[f"e1_{br}_src"]).astype(np.int64),
            e1d=np.asarray(inputs[f"e1_{br}_dst"]).astype(np.int64),
        )

    iota = np.tile(np.arange(128, dtype=np.float32), (128, 1))
    ident = np.eye(128, dtype=np.float32)

    # ---- launch A prep ----
    ect0 = np.ones(T1, np.int64)
    shardinfo = {}
    for br in ("sim", "cor"):
        lists, ect = _shard_prep(branches[br]["e0s"], branches[br]["e0d"], S1, T1)
        shardinfo[br] = lists
        ect0 = np.maximum(ect0, ect)
    nc_a = _build_launch_a(ect0)
    in_maps = []
    for c in range(NC):
        im = {"iota": iota, "ident": ident}
        for br in ("sim", "cor"):
            bb = branches[br]
            es, ed = shardinfo[br][c]
            eidx, dstl = _pack_edges(es, ed, S1, T1, ect0)
            recip, mask1p = _recips(ed, S1, T1)
            gself = np.zeros((T1 * 128, DG), bf16)
            lo = c * S1
            hi = min(lo + T1 * 128, N0)
            gself[:hi - lo] = bb["gtab"][lo:hi]
            im.update({
                f"gtab_{br}": bb["gtab"], f"eidx_{br}": eidx, f"dstl_{br}": dstl,
                f"recip_{br}": recip, f"mask1p_{br}": mask1p,
                f"win_{br}": bb["win"], f"wout_{br}": bb["wout"],
                f"gself_{br}": np.ascontiguousarray(
                    gself.reshape(T1, 128, DG).transpose(1, 0, 2).reshape(128, T1 * DG)),
            })
        in_maps.append(im)
    if trace:
        import shutil
        shutil.rmtree("/root/problem/work/trace_A", ignore_errors=True)
        os.makedirs("/root/problem/work/trace_A", exist_ok=True)
        tkw = {"tmpdir": "/root/problem/work/trace_A"}
    res_a = run_bass_kernel_spmd(nc_a, in_maps, core_ids=list(range(NC)),
                                 trace=trace, **tkw)
    if trace:
        _exec_times.append(res_a.exec_time_ns)

    # ---- host exchange: assemble p [N1, OUT] per branch ----
    ptabs = {}
    for br in ("sim", "cor"):
        cols = []
        for c in range(NC):
            pt = np.asarray(res_a.results[c][f"pt_{br}"])  # [128, T1*128]
            p_shard = pt.T[:S1]                             # [6250, 128]
            cols.append(p_shard)
        ptabs[br] = np.ascontiguousarray(np.concatenate(cols, 0), dtype=np.float32)

    # ---- launch B prep ----
    a1 = float(np.asarray(inputs["a1"]).ravel()[0])
    a2 = float(np.asarray(inputs["a2"]).ravel()[0])
    b2 = float(np.asarray(inputs["b2"]).ravel()[0])
    ect1 = np.ones(T2, np.int64)
    shardinfo1 = {}
    for br in ("sim", "cor"):
        lists, ect = _shard_prep(branches[br]["e1s"], branches[br]["e1d"], S2, T2)
        shardinfo1[br] = lists
        ect1 = np.maximum(ect1, ect)
    nc_b = _build_launch_b(ect1, (a1, a2, b2))
    ones = np.ones((1, 128), np.float32)
    bo = np.zeros((128, 2), np.float32)
    bo[:, 0] = branches["sim"]["bout"]
    bo[:, 1] = branches["cor"]["bout"]
    in_maps = []
    for c in range(NC):
        im = {"iota": iota, "ones": ones, "bo": bo,
              "wcs": np.asarray(inputs["W_cor2sim"], np.float32),
              "wsc": np.asarray(inputs["W_sim2cor"], np.float32)}
        for br in ("sim", "cor"):
            es, ed = shardinfo1[br][c]
            eidx, dstl = _pack_edges(es, ed, S2, T2, ect1)
            cnt_full = np.bincount(ed, minlength=T2 * 128).astype(np.float32)
            recip_full = np.concatenate(
                [1.0 / np.maximum(cnt_full, 1.0), [1.0]]).astype(np.float32)
            # dstl holds tile-local dst (-1 pad); reconstruct global local id per slot
            tcol = np.repeat(np.arange(T2), ect1)[None, :]
            gl = np.where(dstl >= 0, dstl + tcol * 128, T2 * 128).astype(np.int64)
            rpe = recip_full[gl].astype(np.float32)
            ptself = np.zeros((T2 * 128, OUT), np.float32)
            lo = c * S2
            hi = min(lo + T2 * 128, N1)
            ptself[:hi - lo] = ptabs[br][lo:hi]
            im.update({
                f"ptab_{br}": ptabs[br], f"eidx_{br}": eidx, f"dstl_{br}": dstl,
                f"rpe_{br}": rpe,
                f"ptself_{br}": np.ascontiguousarray(
                    ptself.reshape(T2, 128, OUT).transpose(2, 0, 1).reshape(OUT, T2 * 128)),
            })
        in_maps.append(im)
    if trace:
        import shutil
        shutil.rmtree("/root/problem/work/trace_B", ignore_errors=True)
        os.makedirs("/root/problem/work/trace_B", exist_ok=True)
        tkw = {"tmpdir": "/root/problem/work/trace_B"}
    res_b = run_bass_kernel_spmd(nc_b, in_maps, core_ids=list(range(NC)),
                                 trace=trace, **tkw)
    if trace:
        _exec_times.append(res_b.exec_time_ns)

    outs = {}
    for br in ("sim", "cor"):
        rows = []
        for c in range(NC):
            zt = np.asarray(res_b.results[c][f"zt_{br}"])   # [128, T2*128]
            rows.append(zt.T[:S2])
        outs[br] = np.ascontiguousarray(np.concatenate(rows, 0), dtype=np.float32)
    return outs["sim"], outs["cor"]



# revision 4
# speedup vs baseline: 1.4116x; 1.4116x over previous
"""GraphSAGE sim/cor dual-branch GNN on 8 Trainium2 NeuronCores.

Sharding: dst-node partition across 8 cores. Host does index preprocessing
only (edge bucketing, sort, padding, count/reciprocal tables, dtype packing);
all FP tensor compute runs on device via Bass/Tile kernels.

Math reformulation (linearity of mean-aggregation):
  layer0: u[d] = g[d] + mean_{e->d} g[src_e]; h0 = relu(u @ W_in + b_in*(1+[cnt>0]))
  layer1: out[d] = p[d] + mean_{e->d} p[src_e] + b_out, with p = h0 @ W_out
  mixing folded into combined weights: z2sim = sim@Wm1s + cor@Wm2s (etc.)

v3: per-edge rows fetched with the custom InstDMAGatherAnt ucode (one
instruction per (tile-group, src-range) instead of one indirect DMA per 128
edges - SWDGE fixed cost dominated before). int16 gather indices require
splitting the source table into 32768-row ranges; edges are blocked per
(dst-tile, src-range) and padded to 128-slot chunks. One-hot selection
matrices for a whole tile are built in a single wide DVE op.
"""
import os
import numpy as np
import ml_dtypes

N0, N1, N2 = 200000, 50000, 10000
HID, OUT = 256, 128
DG = 144          # concat embedding dim
DGA = DG + 1      # augmented with bias/mask column
GP = 256          # padded gtab row stride (elems) -> 512B, 256-divisible
NC = 8
S1 = N1 // NC     # 6250 dst per core, layer0
S2 = N2 // NC     # 1250 dst per core, layer1
T1 = (S1 + 127) // 128   # 49 tiles
T2 = (S2 + 127) // 128   # 10 tiles
RS = 15           # range shift: 32768 rows per int16-addressable range
RA = (N0 + (1 << RS) - 1) >> RS   # 7 ranges, layer0
RB = (N1 + (1 << RS) - 1) >> RS   # 2 ranges, layer1
GTA = 7           # tiles per group, launch A (49 = 7*7)
GTB = 5           # tiles per group, launch B (10 = 2*5)

_exec_times = []


# ---------------------------------------------------------------- host prep

def _shard_lists(e_src, e_dst, shard):
    lists = []
    for c in range(NC):
        m = (e_dst >= c * shard) & (e_dst < (c + 1) * shard)
        lists.append((e_src[m], e_dst[m] - c * shard))
    return lists


def _ect_blocks(lists, n_tiles, n_ranges):
    """Per-branch chunk counts ect[t, r] = max over cores of
    ceil(#edges(dst-tile t, src-range r) / 128), min 1."""
    ect = np.ones((n_tiles, n_ranges), np.int64)
    for s, d in lists:
        t = d // 128
        r = s >> RS
        cnt = np.zeros((n_tiles, n_ranges), np.int64)
        np.add.at(cnt, (t, r), 1)
        ect = np.maximum(ect, (cnt + 127) // 128)
    return ect


class _Maps:
    """Compile-time layout maps derived from ect (shared by host pack and
    device builder)."""

    def __init__(self, ect, gt):
        T, R = ect.shape
        self.T, self.R, self.gt = T, R, gt
        self.ect = ect
        self.nch = ect.sum(1)                          # chunks per tile
        self.tstart = np.concatenate([[0], np.cumsum(self.nch)]).astype(int)
        self.totc = int(self.nch.sum())
        self.groups = [list(range(g, min(g + gt, T)))
                       for g in range(0, T, gt)]
        # span chunk counts per (group, range)
        self.spanc = [[int(ect[tl, r].sum()) for r in range(R)]
                      for tl in self.groups]
        self.gchunks = [sum(sc) for sc in self.spanc]
        # global slot-chunk offset of span (g, r): groups major, then r
        self.spanstart = []
        acc = 0
        for gi in range(len(self.groups)):
            row = []
            for r in range(R):
                row.append(acc)
                acc += self.spanc[gi][r]
            self.spanstart.append(row)
        assert acc == self.totc
        # within-group column offset of span (g, r)
        self.spancol = [np.concatenate([[0], np.cumsum(sc)]).astype(int)
                        for sc in self.spanc]

    def block_span_col(self, gi, t, r):
        """Column (within group gi's stage) where block (t, r) starts."""
        tl = self.groups[gi]
        off = self.spancol[gi][r]
        for t2 in tl:
            if t2 == t:
                break
            off += int(self.ect[t2, r])
        return int(off)

    def gcols_for_tile(self, gi, t):
        """For tile t's chunks in t-major order, the stage column of each."""
        cols = []
        for r in range(self.R):
            base = self.block_span_col(gi, t, r)
            for j in range(int(self.ect[t, r])):
                cols.append(base + j)
        return cols

    def block_slot0(self, gi, t, r):
        """Global slot index where block (t, r) starts (span order)."""
        col = self.spanstart[gi][r] + self.block_span_col(gi, t, r) \
            - self.spancol[gi][r]
        return col * 128


def _pack_core(src, dstl, maps):
    """Per-core pack: span-ordered int16 gather indices (wrapped 16) and
    t-major dstl [-1 pad] bf16."""
    T, R = maps.T, maps.R
    t_of = dstl // 128
    r_of = src >> RS
    order = np.lexsort((r_of, t_of))
    s_s, d_s, t_s, r_s = src[order], dstl[order], t_of[order], r_of[order]
    key = t_s * R + r_s
    bounds = np.searchsorted(key, np.arange(T * R + 1))
    tot_slots = maps.totc * 128
    idx_flat = np.zeros(tot_slots, np.int16)
    dstl_flat = np.full(maps.totc * 128, -1.0, np.float32)
    for gi, tl in enumerate(maps.groups):
        for t in tl:
            tcum = maps.tstart[t]
            for r in range(R):
                k = t * R + r
                a, b = int(bounds[k]), int(bounds[k + 1])
                n = b - a
                ecr = int(maps.ect[t, r])
                assert n <= ecr * 128
                if n:
                    s0 = maps.block_slot0(gi, t, r)
                    idx_flat[s0:s0 + n] = (s_s[a:b] - (r_s[a:b] << RS)
                                           ).astype(np.int16)
                    c0 = tcum * 128
                    dstl_flat[c0:c0 + n] = (d_s[a:b] - t * 128).astype(
                        np.float32)
                tcum += ecr
    idx_wrap = np.tile(idx_flat.reshape(-1, 16).T, (8, 1)).astype(np.int16)
    dstl = dstl_flat.reshape(maps.totc, 128).T
    return np.ascontiguousarray(idx_wrap), \
        np.ascontiguousarray(dstl.astype(ml_dtypes.bfloat16))


def _recips(dst_local, n_tiles):
    cnt = np.bincount(dst_local, minlength=n_tiles * 128).astype(np.float32)
    recip = 1.0 / np.maximum(cnt, 1.0)
    mask1p = 1.0 + (cnt > 0)
    return (recip.reshape(n_tiles, 128).T.copy(),
            mask1p.astype(np.float32).reshape(n_tiles, 128).T.copy())


# ------------------------------------------------------------ device helper

def _dma_gather_raw(gp, out_ap, in_ap, idxs_ap, num_idxs, elem_size,
                    elem_step):
    """bass dma_gather minus the elem_size%256 assert (non-transpose, HBM
    src). Row stride (elem_step) must still be 256B-divisible."""
    import concourse.mybir as mybir
    import concourse.ap_utils as ap_utils
    stride_bytes = elem_step * mybir.dt.size(in_ap.dtype)
    assert stride_bytes % 256 == 0
    assert in_ap.ap[0][0] == elem_step
    assert in_ap.ap[-1][1] == elem_size
    assert ap_utils.ap_is_contiguous(out_ap.ap[1:])
    assert ap_utils.ap_is_contiguous(idxs_ap.ap[1:])
    _in_ap = gp.lower_ap_dma(in_ap, for_custom_bir_dma=True)
    _idxs_ap = gp.lower_ap(idxs_ap)
    _out_ap = gp.lower_ap(out_ap)
    return gp.add_instruction(
        mybir.InstDMAGatherAnt(
            name=gp.bass.get_next_instruction_name(),
            ins=[*_in_ap, _idxs_ap, gp.lower_val_access(gp.to_reg(num_idxs))],
            outs=[_out_ap],
            transpose=False,
            num_idxs=num_idxs,
            elem_size=elem_size,
            stride_bytes_256=stride_bytes // 256,
            gen_mode=0,
            # single_packet coalesces an engine's whole desc stream into one
            # packet; >64 descs/packet exceeds the SDMA ceiling and hangs.
            single_packet=False,
            queue_num=0,
            sbuf_tokens_per_rank=0,
            sbuf_free_dim_per_rank=0,
            sbuf_free_dim_pad_per_rank=0,
            sbuf_byte_offset=0,
        ))


# ------------------------------------------------------------ launch A

def _build_launch_a(maps):
    import concourse.bacc as bacc
    import concourse.mybir as mybir
    import concourse.tile as tile

    bf16 = mybir.dt.bfloat16
    f32 = mybir.dt.float32
    nc = bacc.Bacc(enable_partition_id=False)
    maxnch = max(int(m.nch.max()) for m in maps.values())
    g = {}
    for br in ("sim", "cor"):
        m = maps[br]
        g[br] = dict(
            gtab=nc.dram_tensor(f"gtab_{br}", [N0, GP], bf16,
                                kind="ExternalInput"),
            idx=nc.dram_tensor(f"idx_{br}", [128, m.totc * 8],
                               mybir.dt.int16, kind="ExternalInput"),
            dstl=nc.dram_tensor(f"dstl_{br}", [128, m.totc], bf16,
                                kind="ExternalInput"),
            recip=nc.dram_tensor(f"recip_{br}", [128, T1], f32,
                                 kind="ExternalInput"),
            mask1p=nc.dram_tensor(f"mask1p_{br}", [128, T1], f32,
                                  kind="ExternalInput"),
            win=nc.dram_tensor(f"win_{br}", [DGA, HID], bf16,
                               kind="ExternalInput"),
            wout=nc.dram_tensor(f"wout_{br}", [HID, OUT], bf16,
                                kind="ExternalInput"),
            gself=nc.dram_tensor(f"gself_{br}", [128, T1 * DG], bf16,
                                 kind="ExternalInput"),
            pt=nc.dram_tensor(f"pt_{br}", [128, T1 * 128], f32,
                              kind="ExternalOutput"),
        )
    iota3_in = nc.dram_tensor("iota3", [128, maxnch * 128], bf16,
                              kind="ExternalInput")
    ident_in = nc.dram_tensor("ident", [128, 128], f32, kind="ExternalInput")

    with tile.TileContext(nc) as tc:
        with tc.tile_pool(name="const", bufs=1) as cp, \
             tc.tile_pool(name="stg", bufs=2) as sp, \
             tc.tile_pool(name="ohp", bufs=3) as ohp, \
             tc.tile_pool(name="work", bufs=3) as wp, \
             tc.tile_pool(name="pagg", bufs=2, space="PSUM") as pagg, \
             tc.tile_pool(name="pfc", bufs=1, space="PSUM") as pfc:
            iota3 = cp.tile([128, maxnch, 128], bf16)
            nc.sync.dma_start(out=iota3[:, :, :], in_=iota3_in[:])
            ident = cp.tile([128, 128], f32)
            nc.sync.dma_start(out=ident[:], in_=ident_in[:])
            for br in ("sim", "cor"):
                tt = g[br]
                m = maps[br]
                idx_t = cp.tile([128, m.totc * 8], mybir.dt.int16,
                                tag=f"ix{br}")
                nc.sync.dma_start(out=idx_t[:], in_=tt["idx"][:])
                dstl_t = cp.tile([128, m.totc], bf16, tag=f"dl{br}")
                nc.sync.dma_start(out=dstl_t[:], in_=tt["dstl"][:])
                recip_t = cp.tile([128, T1], f32, tag=f"rc{br}")
                nc.sync.dma_start(out=recip_t[:], in_=tt["recip"][:])
                mask_t = cp.tile([128, T1], f32, tag=f"mk{br}")
                nc.sync.dma_start(out=mask_t[:], in_=tt["mask1p"][:])
                win_t = cp.tile([128, 2 * HID], bf16, tag=f"wi{br}")
                nc.sync.dma_start(out=win_t[:, :HID], in_=tt["win"][0:128, :])
                nc.sync.dma_start(out=win_t[:DGA - 128, HID:],
                                  in_=tt["win"][128:DGA, :])
                wout_t = cp.tile([128, 2 * OUT], bf16, tag=f"wo{br}")
                nc.sync.dma_start(out=wout_t[:, :OUT], in_=tt["wout"][0:128, :])
                nc.sync.dma_start(out=wout_t[:, OUT:],
                                  in_=tt["wout"][128:HID, :])
                gself_t = cp.tile([128, T1 * DG], bf16, tag=f"gs{br}")
                nc.sync.dma_start(out=gself_t[:], in_=tt["gself"][:])

                maxg = max(m.gchunks)
                for gi, tl in enumerate(m.groups):
                    stage = sp.tile([128, maxg, DG], bf16, tag="stage")
                    for r in range(m.R):
                        sc = m.spanc[gi][r]
                        if sc == 0:
                            continue
                        rlo = r << RS
                        rhi = min(rlo + (1 << RS), N0)
                        col = m.spancol[gi][r]
                        icol = (m.spanstart[gi][r] * 128) // 16
                        _dma_gather_raw(
                            nc.gpsimd,
                            stage[:, col:col + sc, :],
                            tt["gtab"][rlo:rhi, 0:DG],
                            idx_t[:, icol:icol + sc * 8],
                            sc * 128, DG, GP)
                    for t in tl:
                        nch = int(m.nch[t])
                        ts = int(m.tstart[t])
                        oh = ohp.tile([128, maxnch, 128], bf16, tag="oh")
                        nc.vector.tensor_tensor(
                            out=oh[:, :nch, :],
                            in0=dstl_t[:, ts:ts + nch].to_broadcast(
                                [128, nch, 128]),
                            in1=iota3[:, :nch, :],
                            op=mybir.AluOpType.is_equal)
                        agg = pagg.tile([128, DG], f32, tag="agg")
                        gcols = m.gcols_for_tile(gi, t)
                        for j in range(nch):
                            nc.tensor.matmul(
                                agg[:], lhsT=oh[:, j, :],
                                rhs=stage[:, gcols[j], :],
                                start=(j == 0), stop=(j == nch - 1))
                        u = wp.tile([128, DGA], f32, tag="u")
                        nc.vector.tensor_scalar_mul(u[:, :DG], agg[:],
                                                    recip_t[:, t:t + 1])
                        nc.vector.tensor_add(u[:, :DG], u[:, :DG],
                                             gself_t[:, t * DG:(t + 1) * DG])
                        nc.vector.tensor_copy(out=u[:, DG:DGA],
                                              in_=mask_t[:, t:t + 1])
                        uta_p = pfc.tile([128, 128], f32, tag="uta")
                        nc.tensor.transpose(out=uta_p[:], in_=u[:, :128],
                                            identity=ident[:])
                        utb_p = pfc.tile([32, 128], f32, tag="utb")
                        nc.tensor.transpose(out=utb_p[:DGA - 128, :],
                                            in_=u[:, 128:DGA],
                                            identity=ident[:])
                        uta = wp.tile([128, 128], bf16, tag="uta_s")
                        nc.vector.tensor_copy(out=uta[:], in_=uta_p[:])
                        utb = wp.tile([32, 128], bf16, tag="utb_s")
                        nc.vector.tensor_copy(out=utb[:DGA - 128, :],
                                              in_=utb_p[:DGA - 128, :])
                        h0 = wp.tile([128, 2 * 128], bf16, tag="h0")
                        for half in range(2):
                            fc = pfc.tile([128, 128], f32, tag=f"fc{half}")
                            nc.tensor.matmul(
                                fc[:],
                                lhsT=win_t[:, half * 128:half * 128 + 128],
                                rhs=uta[:], start=True, stop=False)
                            nc.tensor.matmul(
                                fc[:],
                                lhsT=win_t[:DGA - 128,
                                           HID + half * 128:
                                           HID + half * 128 + 128],
                                rhs=utb[:DGA - 128, :],
                                start=False, stop=True)
                            nc.scalar.activation(
                                out=h0[:, half * 128:(half + 1) * 128],
                                in_=fc[:],
                                func=mybir.ActivationFunctionType.Relu)
                        pt_p = pfc.tile([128, 128], f32, tag="pt")
                        nc.tensor.matmul(pt_p[:], lhsT=wout_t[:, :OUT],
                                         rhs=h0[:, :128],
                                         start=True, stop=False)
                        nc.tensor.matmul(pt_p[:], lhsT=wout_t[:, OUT:],
                                         rhs=h0[:, 128:],
                                         start=False, stop=True)
                        pt_s = wp.tile([128, 128], f32, tag="pt_s")
                        nc.scalar.mul(pt_s[:], pt_p[:], 1.0)
                        nc.sync.dma_start(
                            out=tt["pt"][:, t * 128:(t + 1) * 128],
                            in_=pt_s[:])
    nc.compile()
    return nc


# ------------------------------------------------------------ launch B

def _build_launch_b(maps, coef):
    import concourse.bacc as bacc
    import concourse.mybir as mybir
    import concourse.tile as tile

    bf16 = mybir.dt.bfloat16
    f32 = mybir.dt.float32
    nc = bacc.Bacc(enable_partition_id=False)
    a1, a2, b2 = coef
    c_self = float(1 - a2 - b2)
    c_cross = float(a2 + b2 * (1 - a1))
    c_ww = float(b2 * a1)
    maxnch = max(int(m.nch.max()) for m in maps.values())
    g = {}
    for br in ("sim", "cor"):
        m = maps[br]
        g[br] = dict(
            ptab=nc.dram_tensor(f"ptab_{br}", [N1, OUT], bf16,
                                kind="ExternalInput"),
            idx=nc.dram_tensor(f"idx_{br}", [128, m.totc * 8],
                               mybir.dt.int16, kind="ExternalInput"),
            dstl=nc.dram_tensor(f"dstl_{br}", [128, m.totc], bf16,
                                kind="ExternalInput"),
            rpe=nc.dram_tensor(f"rpe_{br}", [128, m.totc], bf16,
                               kind="ExternalInput"),
            ptself=nc.dram_tensor(f"ptself_{br}", [128, T2 * 128], f32,
                                  kind="ExternalInput"),
            zt=nc.dram_tensor(f"zt_{br}", [128, T2 * 128], f32,
                              kind="ExternalOutput"),
        )
    wcs_in = nc.dram_tensor("wcs", [OUT, OUT], f32, kind="ExternalInput")
    wsc_in = nc.dram_tensor("wsc", [OUT, OUT], f32, kind="ExternalInput")
    wcsT_in = nc.dram_tensor("wcsT", [OUT, OUT], f32, kind="ExternalInput")
    wscT_in = nc.dram_tensor("wscT", [OUT, OUT], f32, kind="ExternalInput")
    identc_in = nc.dram_tensor("identc", [128, 128], f32,
                               kind="ExternalInput")
    bo_in = nc.dram_tensor("bo", [128, 2], f32, kind="ExternalInput")
    iota3_in = nc.dram_tensor("iota3", [128, maxnch * 128], bf16,
                              kind="ExternalInput")

    with tile.TileContext(nc) as tc:
        with tc.tile_pool(name="const", bufs=1) as cp, \
             tc.tile_pool(name="stg", bufs=2) as sp, \
             tc.tile_pool(name="ohp", bufs=3) as ohp, \
             tc.tile_pool(name="work", bufs=3) as wp, \
             tc.tile_pool(name="pagg", bufs=2, space="PSUM") as pagg, \
             tc.tile_pool(name="pmix", bufs=1, space="PSUM") as pmix:
            iota3 = cp.tile([128, maxnch, 128], bf16)
            nc.sync.dma_start(out=iota3[:, :, :], in_=iota3_in[:])
            wcs = cp.tile([128, OUT], f32)
            nc.sync.dma_start(out=wcs[:], in_=wcs_in[:])
            wsc = cp.tile([128, OUT], f32)
            nc.sync.dma_start(out=wsc[:], in_=wsc_in[:])
            wcsT = cp.tile([128, OUT], f32)
            nc.sync.dma_start(out=wcsT[:], in_=wcsT_in[:])
            wscT = cp.tile([128, OUT], f32)
            nc.sync.dma_start(out=wscT[:], in_=wscT_in[:])
            identc = cp.tile([128, 128], f32)
            nc.sync.dma_start(out=identc[:], in_=identc_in[:])
            bo = cp.tile([128, 2], f32)
            nc.sync.dma_start(out=bo[:], in_=bo_in[:])

            # mixing-weight prologue:
            #   z2sim = sim @ Wm1s + cor @ Wm2s
            #   z2cor = cor @ Wm1c + sim @ Wm2c
            # Wm1s = c_self*I + c_ww*(Wsc@Wcs), Wm2s = c_cross*Wcs
            wa_p = pmix.tile([128, 128], f32, tag="wa")
            nc.tensor.matmul(wa_p[:], lhsT=wscT[:], rhs=wcs[:],
                             start=True, stop=True)
            wm1s = wp.tile([128, 128], f32, tag="wm1s", bufs=1)
            nc.scalar.mul(wm1s[:], wa_p[:], c_ww)
            nc.vector.tensor_add(wm1s[:], wm1s[:], identc[:])
            wb_p = pmix.tile([128, 128], f32, tag="wb")
            nc.tensor.matmul(wb_p[:], lhsT=wcsT[:], rhs=wsc[:],
                             start=True, stop=True)
            wm1c = wp.tile([128, 128], f32, tag="wm1c", bufs=1)
            nc.scalar.mul(wm1c[:], wb_p[:], c_ww)
            nc.vector.tensor_add(wm1c[:], wm1c[:], identc[:])
            wm2s = wp.tile([128, 128], f32, tag="wm2s", bufs=1)
            nc.scalar.mul(wm2s[:], wcs[:], c_cross)
            wm2c = wp.tile([128, 128], f32, tag="wm2c", bufs=1)
            nc.scalar.mul(wm2c[:], wsc[:], c_cross)

            tiles = {}
            for bi, br in enumerate(("sim", "cor")):
                tt = g[br]
                m = maps[br]
                idx_t = cp.tile([128, m.totc * 8], mybir.dt.int16,
                                tag=f"ix{br}")
                nc.sync.dma_start(out=idx_t[:], in_=tt["idx"][:])
                dstl_t = cp.tile([128, m.totc], bf16, tag=f"dl{br}")
                nc.sync.dma_start(out=dstl_t[:], in_=tt["dstl"][:])
                rpe_t = cp.tile([128, m.totc], bf16, tag=f"rp{br}")
                nc.sync.dma_start(out=rpe_t[:], in_=tt["rpe"][:])
                ptself_t = cp.tile([128, T2 * 128], f32, tag=f"ps{br}")
                nc.sync.dma_start(out=ptself_t[:], in_=tt["ptself"][:])
                # fold b_out into the self term once
                nc.vector.tensor_scalar_add(ptself_t[:], ptself_t[:],
                                            bo[:, bi:bi + 1])
                tiles[br] = (idx_t, dstl_t, rpe_t, ptself_t)

            mA, mB_ = maps["sim"], maps["cor"]
            assert len(mA.groups) == len(mB_.groups)
            for gi in range(len(mA.groups)):
                stages = {}
                for br in ("sim", "cor"):
                    tt = g[br]
                    m = maps[br]
                    idx_t = tiles[br][0]
                    maxg = max(m.gchunks)
                    stage = sp.tile([128, maxg, OUT], bf16, tag=f"st{br}")
                    for r in range(m.R):
                        sc = m.spanc[gi][r]
                        if sc == 0:
                            continue
                        rlo = r << RS
                        rhi = min(rlo + (1 << RS), N1)
                        col = m.spancol[gi][r]
                        icol = (m.spanstart[gi][r] * 128) // 16
                        _dma_gather_raw(
                            nc.gpsimd,
                            stage[:, col:col + sc, :],
                            tt["ptab"][rlo:rhi, 0:OUT],
                            idx_t[:, icol:icol + sc * 8],
                            sc * 128, OUT, OUT)
                    stages[br] = stage
                for ti in range(len(mA.groups[gi])):
                    br_out = {}
                    for br in ("sim", "cor"):
                        m = maps[br]
                        t = m.groups[gi][ti]
                        _, dstl_t, rpe_t, ptself_t = tiles[br]
                        nch = int(m.nch[t])
                        ts = int(m.tstart[t])
                        oh = ohp.tile([128, maxnch, 128], bf16, tag="oh")
                        nc.vector.tensor_tensor(
                            out=oh[:, :nch, :],
                            in0=dstl_t[:, ts:ts + nch].to_broadcast(
                                [128, nch, 128]),
                            in1=iota3[:, :nch, :],
                            op=mybir.AluOpType.is_equal)
                        ohs = ohp.tile([128, maxnch, 128], bf16, tag="ohs")
                        nc.vector.tensor_tensor(
                            out=ohs[:, :nch, :],
                            in0=rpe_t[:, ts:ts + nch].to_broadcast(
                                [128, nch, 128]),
                            in1=oh[:, :nch, :],
                            op=mybir.AluOpType.mult)
                        aggp = pagg.tile([128, 128], f32, tag="aggp")
                        gcols = m.gcols_for_tile(gi, t)
                        for j in range(nch):
                            nc.tensor.matmul(
                                aggp[:], lhsT=stages[br][:, gcols[j], :],
                                rhs=ohs[:, j, :],
                                start=(j == 0), stop=(j == nch - 1))
                        sT = wp.tile([128, 128], f32, tag=f"sT{br}")
                        nc.vector.tensor_add(
                            sT[:], ptself_t[:, t * 128:(t + 1) * 128],
                            aggp[:])
                        br_out[br] = (sT, t)
                    (sT, t), (cT, _) = br_out["sim"], br_out["cor"]
                    zs_p = pmix.tile([128, 128], f32, tag="zs")
                    nc.tensor.matmul(zs_p[:], lhsT=wm1s[:], rhs=sT[:],
                                     start=True, stop=False)
                    nc.tensor.matmul(zs_p[:], lhsT=wm2s[:], rhs=cT[:],
                                     start=False, stop=True)
                    zs = wp.tile([128, 128], f32, tag="zss")
                    nc.scalar.mul(zs[:], zs_p[:], 1.0)
                    nc.sync.dma_start(
                        out=g["sim"]["zt"][:, t * 128:(t + 1) * 128],
                        in_=zs[:])
                    zc_p = pmix.tile([128, 128], f32, tag="zc")
                    nc.tensor.matmul(zc_p[:], lhsT=wm1c[:], rhs=cT[:],
                                     start=True, stop=False)
                    nc.tensor.matmul(zc_p[:], lhsT=wm2c[:], rhs=sT[:],
                                     start=False, stop=True)
                    zc = wp.tile([128, 128], f32, tag="zcs")
                    nc.scalar.mul(zc[:], zc_p[:], 1.0)
                    nc.sync.dma_start(
                        out=g["cor"]["zt"][:, t * 128:(t + 1) * 128],
                        in_=zc[:])
    nc.compile()
    return nc


# ------------------------------------------------------------ orchestration

def kernel(**inputs):
    from concourse.bass_utils import run_bass_kernel_spmd
    global _exec_times
    _exec_times = []
    trace = os.environ.get("BASS_KERNEL_TRACE", "0") == "1"
    tkw = {}
    if trace:
        import sys, types
        import antenv
        from trn_agent_boot.trn_boot import _ntff_profile_via_ctypes
        if "antenv.axon_hooks" not in sys.modules:
            mod = types.ModuleType("antenv.axon_hooks")
            mod.get_axon_ntff_profile_hook = (
                lambda: _ntff_profile_via_ctypes("/opt/axon/libaxon_pjrt.so"))
            mod.set_axon_ntff_profile_hook = lambda h: None
            sys.modules["antenv.axon_hooks"] = mod
            antenv.axon_hooks = mod

    bf16 = ml_dtypes.bfloat16
    x = np.asarray(inputs["x"]).astype(np.int64)
    branches = {}
    for br in ("sim", "cor"):
        tabs = [np.asarray(inputs[f"emb_{br}_{i}"], np.float32)
                for i in range(5)]
        gtab = np.concatenate([tabs[i][x[:, i]] for i in range(5)], axis=1)
        gtabp = np.zeros((N0, GP), bf16)
        gtabp[:, :DG] = gtab.astype(bf16)
        win = np.asarray(inputs[f"W_in_{br}"], np.float32)
        bin_ = np.asarray(inputs[f"b_in_{br}"], np.float32)
        win_aug = np.concatenate([win, bin_[None, :]], 0)
        branches[br] = dict(
            gtab=gtab.astype(bf16),
            gtabp=gtabp,
            win=win_aug.astype(bf16),
            wout=np.asarray(inputs[f"W_out_{br}"], np.float32).astype(bf16),
            bout=np.asarray(inputs[f"b_out_{br}"], np.float32),
            e0s=np.asarray(inputs[f"e0_{br}_src"]).astype(np.int64),
            e0d=np.asarray(inputs[f"e0_{br}_dst"]).astype(np.int64),
            e1s=np.asarray(inputs[f"e1_{br}_src"]).astype(np.int64),
            e1d=np.asarray(inputs[f"e1_{br}_dst"]).astype(np.int64),
        )

    ident = np.eye(128, dtype=np.float32)

    # ---- launch A ----
    mapsA = {}
    shardinfo = {}
    for br in ("sim", "cor"):
        lists = _shard_lists(branches[br]["e0s"], branches[br]["e0d"], S1)
        shardinfo[br] = lists
        mapsA[br] = _Maps(_ect_blocks(lists, T1, RA), GTA)
    maxnchA = max(int(m.nch.max()) for m in mapsA.values())
    iota3A = np.tile(np.arange(128, dtype=np.float32).astype(bf16)[None, :],
                     (128, maxnchA)).astype(bf16)
    nc_a = _build_launch_a(mapsA)
    in_maps = []
    for c in range(NC):
        im = {"iota3": iota3A, "ident": ident}
        for br in ("sim", "cor"):
            bb = branches[br]
            m = mapsA[br]
            es, ed = shardinfo[br][c]
            idx_w, dstl = _pack_core(es, ed, m)
            recip, mask1p = _recips(ed, T1)
            gself = np.zeros((T1 * 128, DG), bf16)
            lo = c * S1
            hi = min(lo + T1 * 128, N0)
            gself[:hi - lo] = bb["gtab"][lo:hi]
            im.update({
                f"gtab_{br}": bb["gtabp"], f"idx_{br}": idx_w,
                f"dstl_{br}": dstl,
                f"recip_{br}": recip, f"mask1p_{br}": mask1p,
                f"win_{br}": bb["win"], f"wout_{br}": bb["wout"],
                f"gself_{br}": np.ascontiguousarray(
                    gself.reshape(T1, 128, DG).transpose(1, 0, 2)
                    .reshape(128, T1 * DG)),
            })
        in_maps.append(im)
    if trace:
        import shutil
        shutil.rmtree("/root/problem/work/trace_A", ignore_errors=True)
        os.makedirs("/root/problem/work/trace_A", exist_ok=True)
        tkw = {"tmpdir": "/root/problem/work/trace_A"}
    res_a = run_bass_kernel_spmd(nc_a, in_maps, core_ids=list(range(NC)),
                                 trace=trace, **tkw)
    if trace:
        _exec_times.append(res_a.exec_time_ns)

    # ---- host exchange ----
    ptabs = {}
    for br in ("sim", "cor"):
        cols = []
        for c in range(NC):
            pt = np.asarray(res_a.results[c][f"pt_{br}"])
            cols.append(pt.T[:S1])
        ptabs[br] = np.ascontiguousarray(np.concatenate(cols, 0),
                                         dtype=np.float32)

    # ---- launch B ----
    a1 = float(np.asarray(inputs["a1"]).ravel()[0])
    a2 = float(np.asarray(inputs["a2"]).ravel()[0])
    b2 = float(np.asarray(inputs["b2"]).ravel()[0])
    mapsB = {}
    shardinfo1 = {}
    for br in ("sim", "cor"):
        lists = _shard_lists(branches[br]["e1s"], branches[br]["e1d"], S2)
        shardinfo1[br] = lists
        mapsB[br] = _Maps(_ect_blocks(lists, T2, RB), GTB)
    maxnchB = max(int(m.nch.max()) for m in mapsB.values())
    iota3B = np.tile(np.arange(128, dtype=np.float32).astype(bf16)[None, :],
                     (128, maxnchB)).astype(bf16)
    nc_b = _build_launch_b(mapsB, (a1, a2, b2))
    identc = ((1.0 - a2 - b2) * np.eye(128)).astype(np.float32)
    bo = np.zeros((128, 2), np.float32)
    bo[:, 0] = branches["sim"]["bout"]
    bo[:, 1] = branches["cor"]["bout"]
    in_maps = []
    for c in range(NC):
        im = {"iota3": iota3B, "identc": identc, "bo": bo,
              "wcs": np.asarray(inputs["W_cor2sim"], np.float32),
              "wsc": np.asarray(inputs["W_sim2cor"], np.float32),
              "wcsT": np.ascontiguousarray(
                  np.asarray(inputs["W_cor2sim"], np.float32).T),
              "wscT": np.ascontiguousarray(
                  np.asarray(inputs["W_sim2cor"], np.float32).T)}
        for br in ("sim", "cor"):
            m = mapsB[br]
            es, ed = shardinfo1[br][c]
            idx_w, dstl = _pack_core(es, ed, m)
            # per-slot reciprocal (t-major slot order must match dstl)
            cnt = np.bincount(ed, minlength=T2 * 128).astype(np.float32)
            recip_full = 1.0 / np.maximum(cnt, 1.0)
            dstl_f = np.asarray(dstl, np.float32)
            tcol = np.zeros(m.totc, np.int64)
            for t in range(T2):
                tcol[m.tstart[t]:m.tstart[t + 1]] = t
            gl = np.where(dstl_f >= 0,
                          dstl_f + tcol[None, :] * 128, 0).astype(np.int64)
            rpe = np.where(dstl_f >= 0, recip_full[gl], 1.0)
            ptself = np.zeros((T2 * 128, OUT), np.float32)
            lo = c * S2
            hi = min(lo + T2 * 128, N1)
            ptself[:hi - lo] = ptabs[br][lo:hi]
            im.update({
                f"ptab_{br}": ptabs[br].astype(bf16), f"idx_{br}": idx_w,
                f"dstl_{br}": dstl, f"rpe_{br}": rpe.astype(bf16),
                f"ptself_{br}": np.ascontiguousarray(
                    ptself.reshape(T2, 128, OUT).transpose(2, 0, 1)
                    .reshape(OUT, T2 * 128)),
            })
        in_maps.append(im)
    if trace:
        import shutil
        shutil.rmtree("/root/problem/work/trace_B", ignore_errors=True)
        os.makedirs("/root/problem/work/trace_B", exist_ok=True)
        tkw = {"tmpdir": "/root/problem/work/trace_B"}
    res_b = run_bass_kernel_spmd(nc_b, in_maps, core_ids=list(range(NC)),
                                 trace=trace, **tkw)
    if trace:
        _exec_times.append(res_b.exec_time_ns)

    outs = {}
    for br in ("sim", "cor"):
        rows = []
        for c in range(NC):
            zt = np.asarray(res_b.results[c][f"zt_{br}"])
            rows.append(zt.T[:S2])
        outs[br] = np.ascontiguousarray(np.concatenate(rows, 0),
                                        dtype=np.float32)
    return outs["sim"], outs["cor"]
